# revision 5
# baseline (speedup 1.0000x reference)
"""Multi-head attention (N=4, L=1024, E=1024, H=16, D=64) on 8 trn2 NeuronCores.

Sharding: core c = (batch n = c//2, head-group g = c%2); each core owns 8 heads
of one batch. Projections + attention + a partial output projection run on
device; the host sums the two per-batch partials and adds the output bias.

Design notes (vs the original baseline; ~166us mean, best reading 164.5us,
run-to-run variance ~+-2.5us):
  - the attention middle is co-paced by the PE matmul stream and ScalarE
    (~71us of exp; only ScalarE has exp), so the k/q projections, the
    score+exp bursts, the v-projection pos-tiles, and the AV/normalize
    blocks are interleaved per head-pair to keep both engines fed;
  - memset-fed warmup matmuls ramp the PE clock during the input-DMA window;
    first input chunks are split so the first projection matmuls start
    sooner; all matrices are host-pre-swizzled to [128, ktiles*width] so
    every input DMA is a long linear per-partition run (~420GB/s observed);
  - softmax denominator: DVE row copy + reciprocal, then
    gpsimd.partition_broadcast of the reciprocal row (no DRAM bounce);
    odd head of each pair is processed first so its cross-partition DMA
    shift overlaps the even head's normalize chain;
  - output projection accumulates all 4 head-pair slices into one PSUM tile
    per 128-query row block (no SBUF accumulator / DVE add chain), with
    m0..2 partials emitted ahead of the m3 dependency to hide pair-3's
    normalize; output partials are bf16, one DMA per row block; PSUM->SBUF
    copies alternate DVE/ScalarE per block.
"""

import sys
from contextlib import ExitStack

sys.path.insert(0, "/opt/trn_rl_repo")

import numpy as np

import concourse.bacc as bacc
import concourse.tile as tile
from concourse import mybir
from concourse.bass_utils import run_bass_kernel_spmd

EMBED = 1024
HEADS = 16
HEAD_DIM = 64
N_BATCH = 4
L = 1024
N_CORES = 8
HG = HEADS // 2          # heads per core
S = HG * HEAD_DIM        # per-core head-slice width (512)
KT = EMBED // 128        # k-tiles over the embed contraction dim (8)
MT = S // 128            # m-tiles over the head-slice dim (4)
F32 = mybir.dt.float32
F32R = mybir.dt.float32r
BF16 = mybir.dt.bfloat16
MM_DTYPE = "bf16"        # "f32r" | "bf16" - dtype of all matmul operands
SCALE = 1.0 / 32.0       # 1/sqrt(EMBED)
DEN_MODE = "gpsimd"      # "gpsimd" | "dma" - softmax-denominator broadcast

_CACHED = {}


def _build(apply_mask: bool):
    MMD = F32R if MM_DTYPE == "f32r" else BF16
    nc = bacc.Bacc("TRN2", target_bir_lowering=False, debug=False,
                   num_devices=N_CORES)

    # host pre-swizzles every matrix to [128, ktiles*width] so DMA
    # descriptors are long linear runs (one per partition per chunk)
    xqT = nc.dram_tensor("xqT", [128, KT * L], MMD, kind="ExternalInput").ap()
    xkT = nc.dram_tensor("xkT", [128, KT * L], MMD, kind="ExternalInput").ap()
    xvT = nc.dram_tensor("xvT", [128, KT * L], MMD, kind="ExternalInput").ap()
    wqT = nc.dram_tensor("wqT", [128, KT * S], MMD, kind="ExternalInput").ap()
    wkT = nc.dram_tensor("wkT", [128, KT * S], MMD, kind="ExternalInput").ap()
    wvT = nc.dram_tensor("wvT", [128, KT * S], MMD, kind="ExternalInput").ap()
    woT = nc.dram_tensor("woT", [128, MT * EMBED], MMD,
                         kind="ExternalInput").ap()
    bq_d = nc.dram_tensor("bq", [128, MT], F32, kind="ExternalInput").ap()
    bk_d = nc.dram_tensor("bk", [128, MT], F32, kind="ExternalInput").ap()
    bv_d = nc.dram_tensor("bv", [1, S], MMD, kind="ExternalInput").ap()
    if apply_mask:
        mb_d = nc.dram_tensor("maskbT", [L, L], F32, kind="ExternalInput").ap()
    out_d = nc.dram_tensor("out_partial", [L, EMBED], BF16,
                           kind="ExternalOutput").ap()

    with tile.TileContext(nc) as tc, ExitStack() as ctx:
        sb = ctx.enter_context(tc.tile_pool(name="sb", bufs=2))
        ps = ctx.enter_context(tc.tile_pool(name="ps", bufs=2, space="PSUM"))
        p2 = ctx.enter_context(tc.tile_pool(name="p2", bufs=2))
        if DEN_MODE == "dma":
            dr = ctx.enter_context(tc.tile_pool(name="dr", bufs=2,
                                                space="DRAM"))

        # ---- memset-fed warmup: ramp the PE clock while input DMAs land
        warm_w = sb.tile([1, 128], MMD, tag="warmw")
        warm_x = sb.tile([1, 512], MMD, tag="warmx")
        ones1 = sb.tile([1, 128], MMD, tag="ones1")
        nc.vector.memset(warm_w[:], 1.0)
        nc.vector.memset(warm_x[:], 1.0)
        nc.vector.memset(ones1[:], 1.0)
        for i in range(12):
            wp = ps.tile([128, 512], F32, tag="pb", bufs=2, name=f"warm{i}")
            nc.tensor.matmul(wp[:], (warm_w[0:1, :]), (warm_x[0:1, :]),
                             start=True, stop=True)

        # ---- input DMA triggers, first-needed first
        bq_sb = sb.tile([128, MT], F32, tag="bias")
        bk_sb = sb.tile([128, MT], F32, tag="bias")
        bv_sb = sb.tile([1, S], MMD, tag="bvrow")

        p1_cm = tc.tile_pool(name="p1", bufs=2)
        p1 = p1_cm.__enter__()

        # DMA triggers issue from BOTH hardware-DGE queues (sync + scalar):
        # each DMA_DIRECT2D occupies its queue ~0.6us, so a single queue
        # serializes the load ramp. Splits of one chunk alternate queues.
        _dma_eng = [nc.sync, nc.scalar]

        def load_chunk(src, width, tag, bufs, nm, c, split=1):
            """Load 4 k-tiles [128, width] (chunk c) of a host-pre-swizzled
            tensor. The host stores [(c k) p w] -> [p, (c k w)] so each DMA
            descriptor is a fully linear multi-KB run per partition.

            split=N issues N sub-chunk DMAs (alternating trigger queues) so
            the first k-tiles land sooner.
            """
            t = p1.tile([128, 4 * width], MMD, tag=tag, bufs=bufs,
                        name=f"{nm}{c}")
            for s_ in range(split):
                fs = slice(s_ * (4 // split) * width,
                           (s_ + 1) * (4 // split) * width)
                off = c * 4 * width
                _dma_eng[s_ % 2].dma_start(
                    t[:, fs], src[:, off + fs.start:off + fs.stop])
            return [t[:, k * width:(k + 1) * width] for k in range(4)]

        wk_t = load_chunk(wkT, S, "w_k", 2, "wk", 0, split=4)
        xk_tiles = load_chunk(xkT, L, "x", 6, "xk", 0, split=4)
        wk_t += load_chunk(wkT, S, "w_k", 2, "wk", 1, split=4)
        xk_tiles += load_chunk(xkT, L, "x", 6, "xk", 1, split=4)
        nc.scalar.dma_start(bk_sb[:], bk_d[:])
        nc.sync.dma_start(bq_sb[:], bq_d[:])
        wq_t, xq_tiles = [], []
        for c in range(2):
            wq_t += load_chunk(wqT, S, "w_q", 2, "wq", c, split=2)
            xq_tiles += load_chunk(xqT, L, "x", 6, "xq", c, split=4)
        nc.scalar.dma_start(bv_sb[:], bv_d[:])
        wv_t, xv_tiles = [], []
        for c in range(2):
            wv_t += load_chunk(wvT, S, "w_v", 2, "wv", c, split=2)
            xv_tiles += load_chunk(xvT, L, "x", 6, "xv", c, split=2)
        wo_all = p2.tile([128, MT * EMBED], MMD, tag="wo", bufs=1, name="wo")
        nc.sync.dma_start(wo_all[:], woT[:])
        wo_t = [wo_all[:, c * EMBED:(c + 1) * EMBED] for c in range(MT)]

        # ---- one m-slice of a projection -> transposed layout [dims, pos].
        # PSUM tag "pb" (not "pa") so later m-slices never wait on the
        # exp-paced score-tile rotation. filler=True sprinkles dependency-
        # free matmuls (into the then-idle "pa" banks) between the DMA-paced
        # accumulation steps of the first m-slices, so input-arrival jitter
        # never idles the PE long enough to trip the HAM clock throttle.
        def proj_m(x_tiles, w_tiles, bias_sb, out_tag, m, filler=False):
            o = sb.tile([128, L], MMD, tag=out_tag, bufs=MT)
            for ch in range(2):
                cs = slice(ch * 512, (ch + 1) * 512)
                p = ps.tile([128, 512], F32, tag="pb", bufs=2)
                for k in range(KT):
                    nc.tensor.matmul(
                        p[:],
                        (w_tiles[k][:, m * 128:(m + 1) * 128]),
                        (x_tiles[k][:, cs]),
                        start=(k == 0), stop=(k == KT - 1))
                    if filler and k % 2 == 1:
                        fp = ps.tile([128, L], F32, tag="pa", bufs=2,
                                     name=f"fill{out_tag}{m}_{ch}_{k}")
                        for _ in range(2):
                            nc.tensor.matmul(fp[:, 0:512], (warm_w[0:1, :]),
                                             (warm_x[0:1, :]),
                                             start=True, stop=True)
                nc.vector.tensor_scalar_add(o[:, cs], p[:],
                                            bias_sb[:, m:m + 1])
            return o

        if apply_mask:
            mb_t = []
            for k in range(KT):
                t = p2.tile([128, L], F32, tag="mb", bufs=KT)
                nc.sync.dma_start(t[:], mb_d[k * 128:(k + 1) * 128, :])
                mb_t.append(t)

        xn_t = [sb.tile([128, L], MMD, tag="xn", bufs=MT, name=f"xn{i}")
                for i in range(MT)]

        def emit_qk_exp(m, kT_m, qT_m, j_order=(0, 1)):
            # k outer / j inner: the two heads' K=64 matmuls are emitted
            # back-to-back so the PE runs them concurrently on row-groups
            # (0,*) and (64,*) (tile_position auto-derives from the 0/64
            # base partitions) -> ~2x effective score-matmul throughput.
            pts = {0: [], 1: []}
            for k in range(KT):
                es = {}
                for j in j_order:
                    h = 2 * m + j
                    rows = slice(j * 64, (j + 1) * 64)
                    e = ps.tile([128, L], F32, tag="pa", bufs=2,
                                name=f"e{h}_{k}")
                    for ch in range(2):
                        cs = slice(ch * 512, (ch + 1) * 512)
                        nc.tensor.matmul(
                            e[:, cs],
                            (kT_m[rows, k * 128:(k + 1) * 128]),
                            (qT_m[rows, cs]),
                            start=True, stop=True)
                    es[j] = e
                for j in j_order:
                    h = 2 * m + j
                    pt = p2.tile([128, L], MMD, tag="pt", bufs=32,
                                 name=f"pt{h}_{k}")
                    if apply_mask:
                        esm = p2.tile([128, L], F32, tag="es", bufs=2,
                                      name=f"es{h}_{k}")
                        nc.vector.tensor_add(esm[:], es[j][:], mb_t[k][:])
                        nc.scalar.activation(
                            pt[:], esm[:],
                            mybir.ActivationFunctionType.Exp, scale=SCALE)
                    else:
                        nc.scalar.activation(
                            pt[:], es[j][:],
                            mybir.ActivationFunctionType.Exp, scale=SCALE)
                    pts[j].append(pt)
            return pts

        def emit_avnorm_head(m, pts, j):
            # chain per chunk: DVE reciprocal on the PSUM den row, gpsimd
            # broadcasts the reciprocal, one DVE multiply normalizes.
            # Both chunks' AV matmuls are interleaved per k-tile so that,
            # when this head's exps are still streaming (last pair), both
            # accumulations chase the exp stream instead of ch1 bulk-waiting
            # behind ch0's k7.
            h = 2 * m + j
            if j == 1:
                xtmp = p2.tile([64, L], MMD, tag="xtmp", bufs=2,
                               name=f"xtmp{h}")
            if DEN_MODE == "dma":
                den = dr.tile([1, L], F32, tag="den", name=f"den{h}")
            # [V|1]^T @ P^T -> numerator rows 0-63, denominator row 64.
            os_ = [ps.tile([65, 512], F32, tag="po", bufs=2,
                           name=f"o{h}_{ch}") for ch in range(2)]
            for k in range(KT):
                for ch in range(2):
                    cs = slice(ch * 512, (ch + 1) * 512)
                    nc.tensor.matmul(os_[ch][:],
                                     (v_t[k][:, h * 65:(h + 1) * 65]),
                                     (pts[j][k][:, cs]),
                                     start=(k == 0), stop=(k == KT - 1))
            for ch in range(2):
                cs = slice(ch * 512, (ch + 1) * 512)
                o = os_[ch]
                den_row = p2.tile([1, 512], F32, tag="denrow",
                                  bufs=2, name=f"denrow{h}_{ch}")
                nc.vector.tensor_copy(den_row[:], o[64:65, :])
                rcp_row = p2.tile([1, 512], F32, tag="rcprow",
                                  bufs=2, name=f"rcprow{h}_{ch}")
                nc.vector.reciprocal_approx_fast(rcp_row[:], den_row[:])
                rcp = p2.tile([64, 512], F32, tag="rcp", bufs=2,
                              name=f"rcp{h}_{ch}")
                if DEN_MODE == "gpsimd":
                    nc.gpsimd.partition_broadcast(rcp[:], rcp_row[:],
                                                  channels=64)
                else:
                    nc.sync.dma_start(den[0:1, cs], rcp_row[:])
                    nc.sync.dma_start(rcp[:],
                                      den[0:1, cs].to_broadcast((64, 512)))
                if j == 0:
                    nc.vector.tensor_mul(xn_t[m][0:64, cs],
                                         o[0:64, :], rcp[:])
                else:
                    nc.vector.tensor_mul(xtmp[:, cs], o[0:64, :], rcp[:])
                    nc.sync.dma_start(xn_t[m][64:128, cs], xtmp[:, cs])

        def emit_avnorm(m, pts):
            # odd head (j=1) first: its cross-partition DMA shift overlaps
            # the even head's normalize chain, shortening the pair's tail.
            emit_avnorm_head(m, pts, 1)
            emit_avnorm_head(m, pts, 0)

        # ---- interleaved schedule: keep the exp stream (ScalarE, ~71us,
        # the middle's near-critical engine) continuously fed while the
        # exp-independent fillers (later projections, vproj) slot between
        # the cheap qk matmul bursts.
        # ---- one v-projection pos-tile -> natural layout [pos, head|ones]
        # (stride 65). Emitted piecewise in the slack slots between qk
        # bursts so ScalarE's exp stream never starves behind a long
        # PE-only vproj lump.
        v_t = []

        def emit_vproj_tile(mp):
            p = ps.tile([128, S], F32, tag="pb", bufs=2)
            for k in range(KT):
                nc.tensor.matmul(p[:], (xv_tiles[k][:, mp * 128:(mp + 1) * 128]),
                                 (wv_t[k]), start=(k == 0), stop=False)
            nc.tensor.matmul(p[:], (ones1[:]), (bv_sb[:]),
                             start=False, stop=True)
            vb = sb.tile([128, HG * 65], MMD, tag="vb", bufs=KT)
            vb3 = vb[:].rearrange("p (h d) -> p h d", h=HG)
            nc.vector.memset(vb3[:, :, 64:65], 1.0)
            nc.vector.tensor_copy(vb3[:, :, 0:64],
                                  p[:].rearrange("p (h d) -> p h d", h=HG))
            v_t.append(vb)

        kT_t, qT_t, ptss = [], [], []
        kT_t.append(proj_m(xk_tiles, wk_t, bk_sb, "kT", 0))
        qT_t.append(proj_m(xq_tiles, wq_t, bq_sb, "qT", 0))
        ptss.append(emit_qk_exp(0, kT_t[0][:], qT_t[0][:]))

        kT_t.append(proj_m(xk_tiles, wk_t, bk_sb, "kT", 1))
        qT_t.append(proj_m(xq_tiles, wq_t, bq_sb, "qT", 1))
        emit_vproj_tile(0)
        emit_vproj_tile(1)
        ptss.append(emit_qk_exp(1, kT_t[1][:], qT_t[1][:]))

        kT_t.append(proj_m(xk_tiles, wk_t, bk_sb, "kT", 2))
        qT_t.append(proj_m(xq_tiles, wq_t, bq_sb, "qT", 2))
        for mp in range(2, KT):
            emit_vproj_tile(mp)
        emit_avnorm(0, ptss[0])
        ptss.append(emit_qk_exp(2, kT_t[2][:], qT_t[2][:]))

        kT_t.append(proj_m(xk_tiles, wk_t, bk_sb, "kT", 3))
        qT_t.append(proj_m(xq_tiles, wq_t, bq_sb, "qT", 3))
        emit_avnorm(1, ptss[1])
        # pair 3: j=1's exp precedes j=0's within each k so the LAST exp is
        # head j=0's k7 -- AV(3,1) chases the stream early, AV(3,0) finishes
        # right after the last exp, and the final normalize (j=0) writes xn
        # directly with no cross-partition DMA shift on the critical tail.
        ptss.append(emit_qk_exp(3, kT_t[3][:], qT_t[3][:], j_order=(1, 0)))

        p1_cm.__exit__(None, None, None)

        emit_avnorm(2, ptss[2])
        emit_avnorm_head(3, ptss[3], 1)

        # ---- output projection: per 128-query block, accumulate all 4
        # head-pair slices into one [128, 1024] PSUM tile (2 matmul groups,
        # one per 512-wide embed half). m0..2 partials are emitted before the
        # m3 dependency so pair 3's normalize latency stays hidden.
        obuf = {}

        def op_partial(qt):
            qs = slice(qt * 128, (qt + 1) * 128)
            f = ps.tile([128, EMBED], F32, tag="pa", bufs=2, name=f"f{qt}")
            for m in range(3):
                for ec in range(2):
                    es_ = slice(ec * 512, (ec + 1) * 512)
                    nc.tensor.matmul(f[:, es_], (xn_t[m][:, qs]),
                                     (wo_t[m][:, es_]),
                                     start=(m == 0), stop=False)
            return f

        def op_finish(qt, f):
            qs = slice(qt * 128, (qt + 1) * 128)
            for ec in range(2):
                es_ = slice(ec * 512, (ec + 1) * 512)
                nc.tensor.matmul(f[:, es_], (xn_t[3][:, qs]),
                                 (wo_t[3][:, es_]),
                                 start=False, stop=True)
            ob = p2.tile([128, EMBED], BF16, tag="obuf", bufs=2,
                         name=f"ob{qt}")
            if qt % 2 == 0:
                nc.vector.tensor_copy(ob[:], f[:])
            else:
                nc.scalar.copy(ob[:], f[:])
            nc.sync.dma_start(out_d[qs, :], ob[:])

        fs = {0: op_partial(0)}
        emit_avnorm_head(3, ptss[3], 0)
        fs[1] = op_partial(1)
        for qt in range(KT):
            op_finish(qt, fs.pop(qt))
            if qt + 2 < KT:
                fs[qt + 2] = op_partial(qt + 2)

    nc.compile()
    return nc


def make_in_maps(values, keys, queries, mask, Wv, bv, Wk, bk, Wq, bq, Wo, bo):
    values = np.asarray(values, dtype=np.float32)
    keys = np.asarray(keys, dtype=np.float32)
    queries = np.asarray(queries, dtype=np.float32)
    mask = np.asarray(mask)
    Wv, bv = np.asarray(Wv, np.float32), np.asarray(bv, np.float32)
    Wk, bk = np.asarray(Wk, np.float32), np.asarray(bk, np.float32)
    Wq, bq = np.asarray(Wq, np.float32), np.asarray(bq, np.float32)
    Wo = np.asarray(Wo, np.float32)

    apply_mask = not bool(np.all(mask != 0))
    if MM_DTYPE == "bf16":
        import ml_dtypes
        mmd_np = ml_dtypes.bfloat16
    else:
        mmd_np = np.float32

    def ct(a):
        return np.ascontiguousarray(np.asarray(a, dtype=np.float32))

    def cm(a):
        return np.ascontiguousarray(np.asarray(a).astype(mmd_np))

    def sw(a):
        # [ktiles*128, w] -> [128, ktiles*w]: linear per-partition DMA runs
        a = np.asarray(a)
        kt, w = a.shape[0] // 128, a.shape[1]
        return cm(a.reshape(kt, 128, w).transpose(1, 0, 2).reshape(128, kt * w))

    in_maps = []
    for c in range(N_CORES):
        n, g = c // 2, c % 2
        sl = slice(g * S, (g + 1) * S)
        m = {
            "xqT": sw(queries[n].T),
            "xkT": sw(keys[n].T),
            "xvT": sw(values[n].T),
            "wqT": sw(Wq[sl, :].T),
            "wkT": sw(Wk[sl, :].T),
            "wvT": sw(Wv[sl, :].T),
            "woT": sw(Wo[:, sl].T),
            "bq": ct(bq[sl].reshape(MT, 128).T),
            "bk": ct(bk[sl].reshape(MT, 128).T),
            "bv": cm(bv[sl].reshape(1, S)),
        }
        if apply_mask:
            mb = np.where(mask[n, 0] == 0, np.float32(-1e20), np.float32(0.0))
            m["maskbT"] = ct(mb.T)
        in_maps.append(m)
    return in_maps, apply_mask


def kernel(values, keys, queries, mask, Wv, bv, Wk, bk, Wq, bq, Wo, bo):
    in_maps, apply_mask = make_in_maps(values, keys, queries, mask, Wv, bv,
                                       Wk, bk, Wq, bq, Wo, bo)
    if apply_mask not in _CACHED:
        _CACHED[apply_mask] = _build(apply_mask)
    nc = _CACHED[apply_mask]

    res = run_bass_kernel_spmd(nc, in_maps, list(range(N_CORES))).results
    bo = np.asarray(bo, np.float32)
    out = np.empty((N_BATCH, L, EMBED), dtype=np.float32)
    for n in range(N_BATCH):
        out[n] = (res[2 * n]["out_partial"].astype(np.float32)
                  + res[2 * n + 1]["out_partial"].astype(np.float32)
                  + bo[None, :])
    return out

